# revision 5
# baseline (speedup 1.0000x reference)
"""Trainium2 Bass kernel for nn_CrAKNVectorAttention2D.

Math: the reference ends with
    weight = softmax(..., axis=-2)            # normalize over j
    out    = einsum('ijk,ik->ik', weight, v)  # = v[i,k] * sum_j weight[i,j,k]
and sum_j softmax(x)[i,j,k] == 1 identically, so the entire pairwise
attention pipeline cancels and out == value == feat @ Wv.T + bv exactly
(up to fp32 rounding of the softmax sum).

The kernel therefore computes value = feat @ Wv.T + bv, data-parallel
over the N=2048 rows across 8 NeuronCores (256 rows/core). Layout is
transposed on host (feat.T), so each core runs a single K=128, M=128,
N=256 fp32 matmul (out_T = Wv @ feat_shard.T in PSUM) and evicts
PSUM -> SBUF with a per-partition bias add on ScalarE.
"""

import numpy as np

N, D = 2048, 128
NCORES = 8
RPC = N // NCORES  # rows per core

TRACE = False
LAST_RESULT = None

_cache = {}


def _install_profile_hook():
    """Restore NTFF profiling under axon: the image's antenv lacks
    axon_hooks, so boot() skipped hook registration. Inject the module
    and register the ctypes-based hook; stub out the artifact upload."""
    if _cache.get("hook_done"):
        return
    _cache["hook_done"] = True
    try:
        import sys
        import types

        import antenv

        if "antenv.axon_hooks" not in sys.modules:
            mod = types.ModuleType("antenv.axon_hooks")
            _hook = [None]
            mod.set_axon_ntff_profile_hook = lambda h: _hook.__setitem__(0, h)
            mod.get_axon_ntff_profile_hook = lambda: _hook[0]
            sys.modules["antenv.axon_hooks"] = mod
            antenv.axon_hooks = mod

        from antenv.axon_hooks import (
            get_axon_ntff_profile_hook,
            set_axon_ntff_profile_hook,
        )

        if get_axon_ntff_profile_hook() is None:
            from trn_agent_boot.trn_boot import _ntff_profile_via_ctypes

            set_axon_ntff_profile_hook(
                _ntff_profile_via_ctypes("/opt/axon/libaxon_pjrt.so")
            )

        import concourse.bass_utils as bu

        bu.upload_artifacts = lambda tmpdir: "local://" + str(tmpdir)
    except Exception as e:  # profiling is best-effort
        print(f"profile hook install failed: {type(e).__name__}: {e}")


PACK = RPC + D + 1  # packed input columns: [featT shard | WvT | bv]


def _get_nc():
    if "nc" in _cache:
        return _cache["nc"]
    import concourse.bacc as bacc
    import concourse.mybir as mybir

    nc = bacc.Bacc("TRN2", target_bir_lowering=False, debug=False)

    pk_dram = nc.dram_tensor("pk", [D, PACK], mybir.dt.float32, kind="ExternalInput").ap()
    outT = nc.dram_tensor("outT", [D, RPC], mybir.dt.float32, kind="ExternalOutput").ap()

    with (
        nc.sbuf_tensor([D, PACK], mybir.dt.float32) as pk,
        nc.sbuf_tensor([D, RPC], mybir.dt.float32) as ot,
        nc.psum_tensor([D, RPC], mybir.dt.float32) as ps,
        nc.semaphore() as dma_sem,
        nc.semaphore() as mm_sem,
        nc.semaphore() as v_sem,
        nc.Block() as block,
    ):

        @block.sync
        def _(sync):
            sync.dma_start(pk[:], pk_dram[:]).then_inc(dma_sem, 16)
            sync.wait_ge(v_sem, 1)
            sync.dma_start(outT[:], ot[:]).then_inc(dma_sem, 16)
            sync.wait_ge(dma_sem, 32)

        @block.tensor
        def _(tensor):
            tensor.wait_ge(dma_sem, 16)
            # out_T[j, n] = sum_k WvT[k, j] * featT[k, n] = (feat @ Wv.T).T
            tensor.matmul(
                ps[:], pk[:, RPC : RPC + D], pk[:, 0:RPC], start=True, stop=True
            ).then_inc(mm_sem, 1)

        @block.vector
        def _(vector):
            vector.wait_ge(mm_sem, 1)
            vector.tensor_scalar_add(ot[:], ps[:], pk[:, RPC + D : PACK]).then_inc(
                v_sem, 1
            )

    nc.compile()
    _cache["nc"] = nc
    return nc


def kernel(**inputs) -> np.ndarray:
    global LAST_RESULT
    from concourse.bass_utils import run_bass_kernel_spmd

    feat = np.ascontiguousarray(np.asarray(inputs["feat"], dtype=np.float32))
    Wv = np.asarray(inputs["Wv"], dtype=np.float32)
    bv = np.asarray(inputs["bv"], dtype=np.float32)

    nc = _get_nc()

    featT = feat.T  # [D, N]
    WvT = Wv.T      # [D, D]; WvT[k, j] = Wv[j, k]

    in_maps = []
    for c in range(NCORES):
        pk = np.empty((D, PACK), dtype=np.float32)
        pk[:, 0:RPC] = featT[:, c * RPC : (c + 1) * RPC]
        pk[:, RPC : RPC + D] = WvT
        pk[:, RPC + D] = bv
        in_maps.append({"pk": pk})
    if TRACE:
        _install_profile_hook()
    res = run_bass_kernel_spmd(nc, in_maps, list(range(NCORES)), trace=TRACE)
    LAST_RESULT = res
    outT = np.concatenate([res.results[c]["outT"] for c in range(NCORES)], axis=1)
    return np.ascontiguousarray(outT.T)


# revision 10
# speedup vs baseline: 1.5875x; 1.5875x over previous
"""Trainium2 Bass kernel for nn_CrAKNVectorAttention2D.

Math: the reference ends with
    weight = softmax(..., axis=-2)            # normalize over j
    out    = einsum('ijk,ik->ik', weight, v)  # = v[i,k] * sum_j weight[i,j,k]
and sum_j softmax(x)[i,j,k] == 1 identically, so the entire pairwise
attention pipeline cancels and out == value == feat @ Wv.T + bv exactly
(up to fp32 rounding of the softmax sum).

The kernel therefore computes value = feat @ Wv.T + bv, data-parallel
over the N=2048 rows across 8 NeuronCores (256 rows/core). Layout is
transposed on host (feat.T), so each core runs a single K=128, M=128,
N=256 fp32 matmul (out_T = Wv @ feat_shard.T in PSUM) and evicts
PSUM -> SBUF with a per-partition bias add on ScalarE.
"""

import numpy as np

N, D = 2048, 128
NCORES = 8
RPC = N // NCORES  # rows per core

TRACE = False
LAST_RESULT = None

_cache = {}


def _install_profile_hook():
    """Restore NTFF profiling under axon: the image's antenv lacks
    axon_hooks, so boot() skipped hook registration. Inject the module
    and register the ctypes-based hook; stub out the artifact upload."""
    if _cache.get("hook_done"):
        return
    _cache["hook_done"] = True
    try:
        import sys
        import types

        import antenv

        if "antenv.axon_hooks" not in sys.modules:
            mod = types.ModuleType("antenv.axon_hooks")
            _hook = [None]
            mod.set_axon_ntff_profile_hook = lambda h: _hook.__setitem__(0, h)
            mod.get_axon_ntff_profile_hook = lambda: _hook[0]
            sys.modules["antenv.axon_hooks"] = mod
            antenv.axon_hooks = mod

        from antenv.axon_hooks import (
            get_axon_ntff_profile_hook,
            set_axon_ntff_profile_hook,
        )

        if get_axon_ntff_profile_hook() is None:
            from trn_agent_boot.trn_boot import _ntff_profile_via_ctypes

            set_axon_ntff_profile_hook(
                _ntff_profile_via_ctypes("/opt/axon/libaxon_pjrt.so")
            )

        import concourse.bass_utils as bu

        bu.upload_artifacts = lambda tmpdir: "local://" + str(tmpdir)
    except Exception as e:  # profiling is best-effort
        print(f"profile hook install failed: {type(e).__name__}: {e}")


PACK = 400  # packed input columns: [featT shard (256) | WvT (128) | bv (1) | pad to 400]
            # 400 cols * 4B = 1600B rows, 64B-aligned for full-rate DMA descriptors


def _get_nc():
    if "nc" in _cache:
        return _cache["nc"]
    import concourse.bacc as bacc
    import concourse.mybir as mybir

    nc = bacc.Bacc(
        "TRN2", target_bir_lowering=False, debug=False, enable_partition_id=False
    )

    pk_dram = nc.dram_tensor("pk", [D, PACK], mybir.dt.float32, kind="ExternalInput").ap()
    outT = nc.dram_tensor("outT", [D, RPC], mybir.dt.float32, kind="ExternalOutput").ap()

    moved = {}

    with (
        nc.sbuf_tensor([D, PACK], mybir.dt.float32) as pk,
        nc.sbuf_tensor([D, RPC], mybir.dt.float32) as ot,
        nc.psum_tensor([D, RPC], mybir.dt.float32) as ps,
        nc.semaphore() as in_sem,
        nc.semaphore() as out_sem,
        nc.semaphore() as mm_sem,
        nc.semaphore() as v_sem,
        nc.Block() as block,
    ):
        # Input DMA on the ACT HWDGE ring; hoisted into `main` post-compile so
        # it issues as early as possible and overlaps the runtime prologue.
        @block.scalar
        def _(scalar):
            moved["dma_in"] = scalar.dma_start(pk[:], pk_dram[:]).then_inc(
                in_sem, 16
            ).ins

        @block.tensor
        def _(tensor):
            tensor.wait_ge(in_sem, 16)
            # out_T[j, n] = sum_k WvT[k, j] * featT[k, n] = (feat @ Wv.T).T
            tensor.matmul(
                ps[:], pk[:, RPC : RPC + D], pk[:, 0:RPC], start=True, stop=True
            ).then_inc(mm_sem, 1)

        @block.vector
        def _(vector):
            vector.wait_ge(mm_sem, 1)
            vector.tensor_scalar_add(ot[:], ps[:], pk[:, RPC + D : RPC + D + 1]).then_inc(
                v_sem, 1
            )

        # Output DMA on the SP HWDGE ring. No completion wait on SP — GpSimd
        # (otherwise idle) carries the completion wait as the NEFF-end
        # sentinel, so the DMA drains concurrently with the other engines'
        # epilogue.
        @block.sync
        def _(sync):
            sync.wait_ge(v_sem, 1)
            sync.dma_start(outT[:], ot[:]).then_inc(out_sem, 16)

        @block.gpsimd
        def _(gpsimd):
            gpsimd.wait_ge(out_sem, 16)  # fuses into Pool's branch

    nc.compile()

    # --- instruction-stream surgery ---
    # All cross-engine dependencies run through explicit semaphores, so the
    # bass entry barrier (incl. unused const-pool memsets) and the end-of-block
    # all-engine barrier are pure overhead: drop them. The walrus-level
    # execution-start/end butterflies still order everything around the kernel.
    import concourse.mybir as mybir

    blocks = nc.m.functions[0].blocks
    main = blocks[0]
    end = next(b for b in blocks if b.name.endswith("_end"))

    def is_barrier_or_memset(ins):
        return type(ins).__name__ in ("InstMemset", "InstDrain", "InstEventSemaphore")

    kept = [i for i in main.instructions if not is_barrier_or_memset(i)]
    removed = len(main.instructions) - len(kept)
    assert removed == 15, f"unexpected main prologue shape: removed {removed}"
    # Hoist the input DMA to the top of `main` so it issues the moment the
    # ACT engine comes up, overlapping the runtime prologue.
    dma_in = moved["dma_in"]
    found = False
    for b in blocks:
        lst = b.instructions
        for i, x in enumerate(lst):
            if x is dma_in:
                del lst[i]
                found = True
                break
        if found:
            break
    assert found, "input DMA not found for hoisting"
    kept = [i for i in kept if i is not dma_in]
    kept.insert(1, dma_in)
    main.instructions[:] = kept

    n_end = len(end.instructions)
    assert n_end == 11, f"unexpected end block shape: {n_end}"
    end.instructions[:] = []

    _cache["nc"] = nc
    return nc


def kernel(**inputs) -> np.ndarray:
    global LAST_RESULT
    from concourse.bass_utils import run_bass_kernel_spmd

    feat = np.ascontiguousarray(np.asarray(inputs["feat"], dtype=np.float32))
    Wv = np.asarray(inputs["Wv"], dtype=np.float32)
    bv = np.asarray(inputs["bv"], dtype=np.float32)

    nc = _get_nc()

    featT = feat.T  # [D, N]
    WvT = Wv.T      # [D, D]; WvT[k, j] = Wv[j, k]

    in_maps = []
    for c in range(NCORES):
        pk = np.zeros((D, PACK), dtype=np.float32)
        pk[:, 0:RPC] = featT[:, c * RPC : (c + 1) * RPC]
        pk[:, RPC : RPC + D] = WvT
        pk[:, RPC + D] = bv
        in_maps.append({"pk": pk})
    if TRACE:
        _install_profile_hook()
    res = run_bass_kernel_spmd(nc, in_maps, list(range(NCORES)), trace=TRACE)
    LAST_RESULT = res
    outT = np.concatenate([res.results[c]["outT"] for c in range(NCORES)], axis=1)
    return np.ascontiguousarray(outT.T)


# revision 11
# speedup vs baseline: 1.5921x; 1.0029x over previous
"""Trainium2 Bass kernel for nn_CrAKNVectorAttention2D.

Math: the reference ends with
    weight = softmax(..., axis=-2)            # normalize over j
    out    = einsum('ijk,ik->ik', weight, v)  # = v[i,k] * sum_j weight[i,j,k]
and sum_j softmax(x)[i,j,k] == 1 identically, so the entire pairwise
attention pipeline cancels and out == value == feat @ Wv.T + bv exactly
(up to fp32 rounding of the softmax sum).

The kernel therefore computes value = feat @ Wv.T + bv, data-parallel
over the N=2048 rows across 8 NeuronCores (256 rows/core). Layout is
transposed on host (feat.T), so each core runs a single K=128, M=128,
N=256 fp32 matmul (out_T = Wv @ feat_shard.T in PSUM) and evicts
PSUM -> SBUF with a per-partition bias add on ScalarE.
"""

import numpy as np

N, D = 2048, 128
NCORES = 8
RPC = N // NCORES  # rows per core

TRACE = False
LAST_RESULT = None

_cache = {}


def _install_profile_hook():
    """Restore NTFF profiling under axon: the image's antenv lacks
    axon_hooks, so boot() skipped hook registration. Inject the module
    and register the ctypes-based hook; stub out the artifact upload."""
    if _cache.get("hook_done"):
        return
    _cache["hook_done"] = True
    try:
        import sys
        import types

        import antenv

        if "antenv.axon_hooks" not in sys.modules:
            mod = types.ModuleType("antenv.axon_hooks")
            _hook = [None]
            mod.set_axon_ntff_profile_hook = lambda h: _hook.__setitem__(0, h)
            mod.get_axon_ntff_profile_hook = lambda: _hook[0]
            sys.modules["antenv.axon_hooks"] = mod
            antenv.axon_hooks = mod

        from antenv.axon_hooks import (
            get_axon_ntff_profile_hook,
            set_axon_ntff_profile_hook,
        )

        if get_axon_ntff_profile_hook() is None:
            from trn_agent_boot.trn_boot import _ntff_profile_via_ctypes

            set_axon_ntff_profile_hook(
                _ntff_profile_via_ctypes("/opt/axon/libaxon_pjrt.so")
            )

        import concourse.bass_utils as bu

        bu.upload_artifacts = lambda tmpdir: "local://" + str(tmpdir)
    except Exception as e:  # profiling is best-effort
        print(f"profile hook install failed: {type(e).__name__}: {e}")


PACK = 400  # packed input columns: [featT shard (256) | WvT (128) | bv (1) | pad to 400]
            # 400 cols * 4B = 1600B rows, 64B-aligned for full-rate DMA descriptors


def _get_nc():
    if "nc" in _cache:
        return _cache["nc"]
    import concourse.bacc as bacc
    import concourse.mybir as mybir

    nc = bacc.Bacc(
        "TRN2", target_bir_lowering=False, debug=False, enable_partition_id=False
    )

    pk_dram = nc.dram_tensor("pk", [D, PACK], mybir.dt.float32, kind="ExternalInput").ap()
    outT = nc.dram_tensor("outT", [D, RPC], mybir.dt.float32, kind="ExternalOutput").ap()

    moved = {}

    with (
        nc.sbuf_tensor([D, PACK], mybir.dt.float32) as pk,
        nc.sbuf_tensor([D, RPC], mybir.dt.float32) as ot,
        nc.psum_tensor([D, RPC], mybir.dt.float32) as ps,
        nc.semaphore() as in_sem,
        nc.semaphore() as out_sem,
        nc.semaphore() as mm_sem,
        nc.semaphore() as v_sem,
        nc.Block() as block,
    ):
        H = RPC // 2

        # Input DMA on the ACT HWDGE ring; hoisted into `main` post-compile so
        # it issues as early as possible and overlaps the runtime prologue.
        # ACT also ships the first output half (its ring is FIFO, so this
        # queues behind the long-finished input DMA).
        @block.scalar
        def _(scalar):
            moved["dma_in"] = scalar.dma_start(pk[:], pk_dram[:]).then_inc(
                in_sem, 16
            ).ins
            scalar.wait_ge(v_sem, 1)
            scalar.dma_start(outT[:, 0:H], ot[:, 0:H]).then_inc(out_sem, 16)

        @block.tensor
        def _(tensor):
            tensor.wait_ge(in_sem, 16)
            # out_T[j, n] = sum_k WvT[k, j] * featT[k, n] = (feat @ Wv.T).T
            tensor.matmul(
                ps[:], pk[:, RPC : RPC + D], pk[:, 0:RPC], start=True, stop=True
            ).then_inc(mm_sem, 1)

        # Bias-add eviction in two halves so each output half's DMA can launch
        # as soon as its half is in SBUF.
        @block.vector
        def _(vector):
            vector.wait_ge(mm_sem, 1)
            bias = pk[:, RPC + D : RPC + D + 1]
            vector.tensor_scalar_add(ot[:, 0:H], ps[:, 0:H], bias).then_inc(v_sem, 1)
            vector.tensor_scalar_add(ot[:, H:RPC], ps[:, H:RPC], bias).then_inc(
                v_sem, 1
            )

        # Second output half on the SP HWDGE ring. No completion wait here —
        # GpSimd (otherwise idle) carries the completion wait as the NEFF-end
        # sentinel, so both DMAs drain concurrently with the other engines'
        # epilogue.
        @block.sync
        def _(sync):
            sync.wait_ge(v_sem, 2)
            sync.dma_start(outT[:, H:RPC], ot[:, H:RPC]).then_inc(out_sem, 16)

        @block.gpsimd
        def _(gpsimd):
            gpsimd.wait_ge(out_sem, 32)  # fuses into Pool's branch

    nc.compile()

    # --- instruction-stream surgery ---
    # All cross-engine dependencies run through explicit semaphores, so the
    # bass entry barrier (incl. unused const-pool memsets) and the end-of-block
    # all-engine barrier are pure overhead: drop them. The walrus-level
    # execution-start/end butterflies still order everything around the kernel.
    import concourse.mybir as mybir

    blocks = nc.m.functions[0].blocks
    main = blocks[0]
    end = next(b for b in blocks if b.name.endswith("_end"))

    def is_barrier_or_memset(ins):
        return type(ins).__name__ in ("InstMemset", "InstDrain", "InstEventSemaphore")

    kept = [i for i in main.instructions if not is_barrier_or_memset(i)]
    removed = len(main.instructions) - len(kept)
    assert removed == 15, f"unexpected main prologue shape: removed {removed}"
    # Hoist the input DMA to the top of `main` so it issues the moment the
    # ACT engine comes up, overlapping the runtime prologue.
    dma_in = moved["dma_in"]
    found = False
    for b in blocks:
        lst = b.instructions
        for i, x in enumerate(lst):
            if x is dma_in:
                del lst[i]
                found = True
                break
        if found:
            break
    assert found, "input DMA not found for hoisting"
    kept = [i for i in kept if i is not dma_in]
    kept.insert(1, dma_in)
    main.instructions[:] = kept

    n_end = len(end.instructions)
    assert n_end == 11, f"unexpected end block shape: {n_end}"
    end.instructions[:] = []

    _cache["nc"] = nc
    return nc


def kernel(**inputs) -> np.ndarray:
    global LAST_RESULT
    from concourse.bass_utils import run_bass_kernel_spmd

    feat = np.ascontiguousarray(np.asarray(inputs["feat"], dtype=np.float32))
    Wv = np.asarray(inputs["Wv"], dtype=np.float32)
    bv = np.asarray(inputs["bv"], dtype=np.float32)

    nc = _get_nc()

    featT = feat.T  # [D, N]
    WvT = Wv.T      # [D, D]; WvT[k, j] = Wv[j, k]

    in_maps = []
    for c in range(NCORES):
        pk = np.zeros((D, PACK), dtype=np.float32)
        pk[:, 0:RPC] = featT[:, c * RPC : (c + 1) * RPC]
        pk[:, RPC : RPC + D] = WvT
        pk[:, RPC + D] = bv
        in_maps.append({"pk": pk})
    if TRACE:
        _install_profile_hook()
    res = run_bass_kernel_spmd(nc, in_maps, list(range(NCORES)), trace=TRACE)
    LAST_RESULT = res
    outT = np.concatenate([res.results[c]["outT"] for c in range(NCORES)], axis=1)
    return np.ascontiguousarray(outT.T)
